# revision 25
# baseline (speedup 1.0000x reference)
# CRF Viterbi decode kernel for Trainium2 (Bass, raw engine programs).
#
# Problem: B=2048 sequences, T=512 steps, K=64 tags, ragged lengths
# (sorted descending). Returns (tag_seq [B,T] int32, tag_scores [B] f32).
#
# Strategy:
#  - Pack two sequences per partition (rank r paired with rank B-1-r) so each
#    partition's total length is ~uniform (~T+eps). 8 cores x 128 partitions
#    hold all 2048 sequences; each core runs one scan of T_pack steps.
#  - Per step t (all engines per core):
#      GPSIMD: ltT[b, j*64+i] = transT[j,i] + f_t[b,j]  (+ end_adj[j] masked
#              per-partition at sequence-end steps)   [bit-exact association
#              (trans + feats) + out, matching the jax reference]
#      DVE:    scores = ltT + out_bcast ; out = grouped-max_i(scores)
#              eq = (scores >= out_bcast_j) * (64-i) ; r = grouped-max(eq)
#              (argmax with first-index tie-break, bitwise-exact)
#      ACT:    bp[t] = uint8(64 - r); also issues feats DMA prefetch (SWDGE,
#              in-order completions)
#  - Backtrace on DVE: tag[t] = bp[t+1][tag[t+1]] via one-hot stt-gather with
#    per-partition end-tag injection at each sequence's last step.
import numpy as np

import concourse.bass as bass
import concourse.mybir as mybir
from concourse import bass_utils

K = 64
PAD_TAG, START_TAG, END_TAG = 0, 1, 2


SELIDX_OP = None
NEG = -10000.0
N_CORES = 8
FBUF = 4   # feats ring buffer depth
PREF = 3   # DMA prefetch distance


def build_program(T):
    """Build the SPMD bass program for T packed time steps."""
    nc = bass.Bass()
    f32 = mybir.dt.float32
    bf16 = mybir.dt.bfloat16
    u8 = mybir.dt.uint8

    # ---- DRAM tensors ----
    # pf[t] = [feats_t (K) | eavec_t (K)] per partition
    pf_d = nc.dram_tensor("pf", [T, 128, K], f32, kind="ExternalInput")
    transT_d = nc.dram_tensor("transT", [128, K * K], f32, kind="ExternalInput")
    iotaR_d = nc.dram_tensor("iotaR", [128, K], f32, kind="ExternalInput")
    iotaRF_d = nc.dram_tensor("iotaRF", [128, K * K], bf16, kind="ExternalInput")
    iota_d = nc.dram_tensor("iota", [128, K], f32, kind="ExternalInput")
    initv_d = nc.dram_tensor("initv", [128, K], f32, kind="ExternalInput")
    emA_d = nc.dram_tensor("emA", [128, T], f32, kind="ExternalInput")
    emB_d = nc.dram_tensor("emB", [128, T], f32, kind="ExternalInput")
    bm_d = nc.dram_tensor("bm", [128, T], f32, kind="ExternalInput")
    ninj_d = nc.dram_tensor("ninj", [128, T], f32, kind="ExternalInput")
    endadj_d = nc.dram_tensor("endadj", [128, K], f32, kind="ExternalInput")
    adjA_d = nc.dram_tensor("adjA", [128, 1], f32, kind="ExternalInput")
    adjB_d = nc.dram_tensor("adjB", [128, 1], f32, kind="ExternalInput")
    emAU_d = nc.dram_tensor("emAU", [128, T], u8, kind="ExternalInput")
    emBU_d = nc.dram_tensor("emBU", [128, T], u8, kind="ExternalInput")
    tags_d = nc.dram_tensor("tags", [128, T], f32, kind="ExternalOutput")
    ts2_d = nc.dram_tensor("ts2", [128, 2], f32, kind="ExternalOutput")

    AL = mybir.AluOpType
    AX = mybir.AxisListType

    from contextlib import ExitStack
    ctx = ExitStack()
    def sb(name, shape, dt):
        return ctx.enter_context(nc.sbuf_tensor(name, shape, dt))
    def sem(name):
        return ctx.enter_context(nc.semaphore(name))
    NLT = 3   # ltT ring depth
    with ctx:
        s_transT = sb("s_transT", [128, K * K], f32)
        lt_bufs = [sb(f"s_lt{i}", [128, K * K], f32) for i in range(NLT)]
        s_sc = sb("s_sc", [128, K * K], f32)
        s_eqb = sb("s_eqb", [128, K * K], bf16)
        s_selb = sb("s_selb", [128, K * K], bf16)
        s_selb2 = sb("s_selb2", [128, K * K // 2], bf16)
        s_iotaRF = sb("s_iotaRF", [128, K * K], bf16)
        s_fb = sb("s_fb", [128, FBUF * K], f32)
        s_endadj = sb("s_endadj", [128, K], f32)
        s_adjA = sb("s_adjA", [128, 1], f32)
        s_adjB = sb("s_adjB", [128, 1], f32)
        s_bp = sb("s_bp", [128, (T + 1) * K], u8)
        s_tags = sb("s_tags", [128, T + 1], f32)
        s_emA = sb("s_emA", [128, T], f32)
        s_emB = sb("s_emB", [128, T], f32)
        s_bm = sb("s_bm", [128, T], f32)
        s_nbm = sb("s_nbm", [128, T], f32)
        s_ninj = sb("s_ninj", [128, T], f32)
        s_injval = sb("s_injval", [128, T], f32)
        s_emAU = sb("s_emAU", [128, T], u8)
        s_emBU = sb("s_emBU", [128, T], u8)
        s_iotaR = sb("s_iotaR", [128, K], f32)
        s_iota = sb("s_iota", [128, K], f32)
        s_initv = sb("s_initv", [128, K], f32)
        s_out = sb("s_out", [128, K], f32)
        s_r = sb("s_r", [128, K], f32)
        s_snapA = sb("s_snapA", [128, K], f32)
        s_snapB = sb("s_snapB", [128, K], f32)
        s_gt = sb("s_gt", [128, K], f32)
        s_eq64 = sb("s_eq64", [128, K], f32)
        s_tmp64 = sb("s_tmp64", [128, K], f32)
        s_ts2 = sb("s_ts2", [128, 2], f32)
        s_g = sb("s_g", [128, 1], f32)
        s_rr = sb("s_rr", [128, 1], f32)
        s_etA = sb("s_etA", [128, 1], f32)
        s_etB = sb("s_etB", [128, 1], f32)
        dma_in = sem("dma_in")
        dma_fs = [sem(f"dma_f{i}") for i in range(FBUF)]
        s_f = sem("s_f")
        s_lt = sem("s_lt")
        s_p1 = sem("s_p1")
        s_p3 = sem("s_p3")
        s_p4 = sem("s_p4")
        s_rs = sem("s_rs")
        s_act = sem("s_act")
        s_done = sem("s_done")
        block = ctx.enter_context(nc.Block())

        N_IN = 15

        def lt3(buf):
            return buf[:].rearrange("p (j i) -> p j i", i=K)

        sc3 = s_sc[:].rearrange("p (j i) -> p j i", i=K)
        eqb3 = s_eqb[:].rearrange("p (j i) -> p j i", i=K)
        selb3 = s_selb[:].rearrange("p (j i) -> p j i", i=K)
        selb2_3 = s_selb2[:].rearrange("p (j i) -> p j i", i=K // 2)
        out_bi = s_out[:].rearrange("p (o i) -> p o i", o=1).broadcast_to([128, K, K])
        m_bj = s_out[:].rearrange("p (j o) -> p j o", o=1).broadcast_to([128, K, K])
        trans3 = s_transT[:].rearrange("p (j i) -> p j i", i=K)

        @block.sync
        def _(sync):
            for sbuf, dr in [
                (s_transT, transT_d), (s_iotaR, iotaR_d), (s_iota, iota_d),
                (s_iotaRF, iotaRF_d), (s_initv, initv_d), (s_emA, emA_d),
                (s_emB, emB_d), (s_bm, bm_d), (s_ninj, ninj_d),
                (s_emAU, emAU_d), (s_emBU, emBU_d),
                (s_endadj, endadj_d), (s_adjA, adjA_d), (s_adjB, adjB_d),
            ]:
                sync.dma_start(sbuf[:], dr[:]).then_inc(dma_in, 16)
            sync.dma_start(s_injval[:], ninj_d[:]).then_inc(dma_in, 16)
            sync.wait_ge(s_done, 1)
            sync.dma_start(tags_d[:], s_tags[:, 0:T]).then_inc(dma_in, 16)
            sync.dma_start(ts2_d[:], s_ts2[:]).then_inc(dma_in, 16)

        @block.scalar
        def _(scalar):
            scalar.wait_ge(dma_in, 16 * N_IN)
            for k in range(1, min(PREF, T - 1) + 1):
                scalar.dma_start(
                    s_fb[:, (k % FBUF) * K:(k % FBUF + 1) * K],
                    pf_d[k]).then_inc(dma_fs[k % FBUF], 16)
            for t in range(1, T):
                k = t + PREF
                if k <= T - 1:
                    if k - FBUF >= 1:
                        scalar.wait_ge(s_f, k - FBUF)
                    scalar.dma_start(
                        s_fb[:, (k % FBUF) * K:(k % FBUF + 1) * K],
                        pf_d[k]).then_inc(dma_fs[k % FBUF], 16)
                scalar.wait_ge(s_rs, t)
                scalar.activation(
                    out=s_bp[:, t * K:(t + 1) * K], in_=s_r[:],
                    func=mybir.ActivationFunctionType.Copy, bias=64.0, scale=-1.0,
                ).then_inc(s_act, 1)

        @block.gpsimd
        def _(gpsimd):
            gpsimd.wait_ge(dma_in, 16 * N_IN)
            for t in range(1, T):
                gpsimd.wait_ge(dma_fs[t % FBUF], 16 * ((t - 1) // FBUF + 1))
                if t > 2:
                    gpsimd.wait_ge(s_p1, t - 2)
                buf = lt_bufs[t % NLT]
                base = (t % FBUF) * K
                fb = s_fb[:, base:base + K]
                f_bj = fb.rearrange("p (j o) -> p j o", o=1).broadcast_to([128, K, K])
                # ltT = transT + f, in two halves aligned into DVE's reduce
                # windows (reduces are single-port and contention-immune)
                gpsimd.tensor_tensor(out=lt3(buf)[:, 2:40, :],
                                     in0=trans3[:, 2:40, :],
                                     in1=f_bj[:, 2:40, :],
                                     op=AL.add).then_inc(s_lt, 1)
                if t > 2:
                    gpsimd.wait_ge(s_p4, t - 2)
                gpsimd.tensor_tensor(out=lt3(buf)[:, 40:64, :],
                                     in0=trans3[:, 40:64, :],
                                     in1=f_bj[:, 40:64, :],
                                     op=AL.add).then_inc(s_lt, 1)
                gpsimd.sem_inc(s_f, 1)
                gpsimd.drain()

        @block.vector
        def _(vector):
            vector.wait_ge(dma_in, 16 * N_IN)
            vector.tensor_copy(s_out[:], s_initv[:])
            vector.memset(s_r[:], 0.0)
            vector.memset(s_snapA[:], 0.0)
            vector.memset(s_snapB[:], 0.0)
            # nbm = 1 - bm
            vector.tensor_scalar(out=s_nbm[:], in0=s_bm[:], scalar1=1.0,
                                 scalar2=-1.0, op0=AL.subtract, op1=AL.mult)
            vector.drain()
            vector.scalar_tensor_tensor(
                out=s_snapA[:], in0=s_out[:], scalar=s_emA[:, 0:1],
                in1=s_snapA[:], op0=AL.mult, op1=AL.add)
            vector.drain()
            for t in range(1, T):
                vector.wait_ge(s_lt, 2 * t)
                buf = lt_bufs[t % NLT]
                # P1: scores = ltT + out_bcast
                vector.tensor_tensor(out=sc3[:, 2:K, :], in0=lt3(buf)[:, 2:K, :],
                                     in1=out_bi[:, 2:K, :],
                                     op=AL.add).then_inc(s_p1, 1)
                vector.drain()
                # P2: out = m = grouped max over i
                vector.tensor_reduce(out=s_out[:, 2:K], in_=sc3[:, 2:K, :],
                                      axis=AX.X, op=AL.max)
                vector.drain()
                # boundary reset: out = out*(1-bm) + init*bm  (boundary
                # partitions' bp this step are don't-care, so P3 may compare
                # against the post-reset state)
                vector.tensor_scalar(out=s_tmp64[:], in0=s_out[:],
                                     scalar1=s_nbm[:, t:t + 1], scalar2=None,
                                     op0=AL.mult)
                vector.drain()
                vector.scalar_tensor_tensor(
                    out=s_out[:], in0=s_initv[:], scalar=s_bm[:, t:t + 1],
                    in1=s_tmp64[:], op0=AL.mult, op1=AL.add)
                vector.drain()
                vector.scalar_tensor_tensor(
                    out=s_snapA[:], in0=s_out[:], scalar=s_emA[:, t:t + 1],
                    in1=s_snapA[:], op0=AL.mult, op1=AL.add)
                vector.scalar_tensor_tensor(
                    out=s_snapB[:], in0=s_out[:], scalar=s_emB[:, t:t + 1],
                    in1=s_snapB[:], op0=AL.mult, op1=AL.add)
                vector.drain()
                # P3: eq = scores >= m (bf16 out)
                vector.tensor_tensor(out=eqb3[:, 2:K, :], in0=sc3[:, 2:K, :],
                                     in1=m_bj[:, 2:K, :],
                                     op=AL.is_ge).then_inc(s_p3, 1)
                if t >= 2:
                    vector.wait_ge(s_act, t - 1)
                vector.drain()
                # P4 (bf16 2x): sel = eq * (64 - i)
                vector.tensor_tensor(out=s_selb[:, 2 * K:], in0=s_eqb[:, 2 * K:],
                                     in1=s_iotaRF[:, 2 * K:],
                                     op=AL.mult).then_inc(s_p4, 1)
                vector.drain()
                # P4b (bf16 2x): fold i-halves with max
                vector.tensor_tensor(out=selb2_3[:, 2:K, :],
                                     in0=selb3[:, 2:K, 0:K // 2],
                                     in1=selb3[:, 2:K, K // 2:K], op=AL.max)
                vector.drain()
                # P5: r = grouped max -> bp = 64 - r
                vector.tensor_reduce(out=s_r[:, 2:K], in_=selb2_3[:, 2:K, :],
                                     op=AL.max, axis=AX.X).then_inc(s_rs, 1)
                vector.drain()

            # ---- apply end adjustment to snapshots (skip length-1 seqs) ----
            vector.scalar_tensor_tensor(
                out=s_snapA[:], in0=s_endadj[:], scalar=s_adjA[:],
                in1=s_snapA[:], op0=AL.mult, op1=AL.add)
            vector.scalar_tensor_tensor(
                out=s_snapB[:], in0=s_endadj[:], scalar=s_adjB[:],
                in1=s_snapB[:], op0=AL.mult, op1=AL.add)
            vector.drain()
            # ---- end-tag extraction ----
            for snap, col, etag in [(s_snapA, 0, s_etA), (s_snapB, 1, s_etB)]:
                vector.tensor_reduce(out=s_ts2[:, col:col + 1], in_=snap[:],
                                     axis=AX.X, op=AL.max)
                vector.drain()
                vector.tensor_tensor(
                    out=s_eq64[:], in0=snap[:],
                    in1=s_ts2[:, col:col + 1].broadcast_to([128, K]), op=AL.is_ge)
                vector.drain()
                vector.tensor_tensor(out=s_eq64[:], in0=s_eq64[:], in1=s_iotaR[:],
                                     op=AL.mult)
                vector.drain()
                vector.tensor_reduce(out=s_rr[:], in_=s_eq64[:], axis=AX.X,
                                     op=AL.max)
                vector.drain()
                vector.tensor_scalar(out=etag[:], in0=s_rr[:], scalar1=64.0,
                                     scalar2=-1.0, op0=AL.subtract, op1=AL.mult)
                vector.drain()

            # ---- injection values ----
            vector.memset(s_injval[:], 0.0)
            vector.drain()
            vector.copy_predicated(s_injval[:], s_emAU[:],
                                   s_etA[:].broadcast_to([128, T]))
            vector.drain()
            vector.copy_predicated(s_injval[:], s_emBU[:],
                                   s_etB[:].broadcast_to([128, T]))
            vector.drain()

            # ---- backtrace ----
            vector.wait_ge(s_act, T - 1)
            vector.memset(s_bp[:, T * K:(T + 1) * K], 0)
            vector.memset(s_tags[:, T:T + 1], 0.0)
            vector.drain()
            for t in range(T - 1, -1, -1):
                vector.scalar_tensor_tensor(
                    out=s_gt[:], in0=s_iota[:], scalar=s_tags[:, t + 1:t + 2],
                    in1=s_bp[:, (t + 1) * K:(t + 2) * K],
                    op0=AL.is_equal, op1=AL.mult, accum_out=s_g[:])
                vector.drain()
                vector.scalar_tensor_tensor(
                    out=s_tags[:, t:t + 1], in0=s_g[:], scalar=s_ninj[:, t:t + 1],
                    in1=s_injval[:, t:t + 1], op0=AL.mult, op1=AL.add)
                vector.drain()
            vector.sem_inc(s_done, 1)

    return nc


def _host_prep(feats, leng, transitions):
    """Pack sequences, build per-core inputs and the shared consts."""
    B, T_in, Kk = feats.shape
    assert Kk == K
    npairs = B // 2
    order = np.argsort(-leng.astype(np.int64), kind="stable")
    a_idx = order[:npairs]
    b_idx = order[B - 1:npairs - 1:-1]  # order[B-1], ..., order[npairs]
    L1 = leng[a_idx].astype(np.int64)
    L2 = leng[b_idx].astype(np.int64)
    T = int((L1 + L2).max())

    # shared consts
    transT = np.ascontiguousarray(transitions.T).reshape(1, K * K)
    transT_b = np.broadcast_to(transT, (128, K * K)).copy()
    iotaR = np.broadcast_to((K - np.arange(K)).astype(np.float32), (128, K)).copy()
    import ml_dtypes
    iotaRF = np.broadcast_to(
        np.tile((K - np.arange(K)).astype(ml_dtypes.bfloat16), K), (128, K * K)).copy()
    iota = np.broadcast_to(np.arange(K).astype(np.float32), (128, K)).copy()
    initv_row = np.full((K,), NEG, np.float32)
    initv_row[START_TAG] = 0.0
    initv = np.broadcast_to(initv_row, (128, K)).copy()
    endadj_row = np.full((K,), NEG, np.float32)
    endadj_row[END_TAG] = 0.0
    endadj = np.broadcast_to(endadj_row, (128, K)).copy()

    n_cores = npairs // 128
    assert npairs % 128 == 0
    in_maps = []
    metas = []
    for c in range(n_cores):
        sl = slice(c * 128, (c + 1) * 128)
        l1 = L1[sl]
        l2 = L2[sl]
        ai = a_idx[sl]
        bi = b_idx[sl]
        pf = np.zeros((T, 128, K), np.float32)
        emA = np.zeros((128, T), np.float32)
        emB = np.zeros((128, T), np.float32)
        bm = np.zeros((128, T), np.float32)
        for q in range(128):
            pf[: l1[q], q, :] = feats[ai[q], : l1[q]]
            pf[l1[q]: l1[q] + l2[q], q, :] = feats[bi[q], : l2[q]]
            emA[q, l1[q] - 1] = 1.0
            emB[q, l1[q] + l2[q] - 1] = 1.0
            bm[q, l1[q]] = 1.0
        emE = emA + emB
        ninj = 1.0 - emE
        adjA = (l1 >= 2).astype(np.float32).reshape(128, 1)
        adjB = (l2 >= 2).astype(np.float32).reshape(128, 1)
        in_maps.append({
            "pf": pf, "transT": transT_b, "iotaR": iotaR, "iota": iota,
            "iotaRF": iotaRF,
            "initv": initv, "emA": emA,
            "emB": emB, "bm": bm, "ninj": ninj, "endadj": endadj,
            "adjA": adjA, "adjB": adjB,
            "emAU": emA.astype(np.uint8), "emBU": emB.astype(np.uint8),
        })
        metas.append((ai, bi, l1, l2))
    return T, in_maps, metas


def _host_post(results, metas, B, T_in):
    tag_seq = np.zeros((B, T_in), np.int32)
    tag_scores = np.zeros((B,), np.float32)
    for c in range(len(metas)):
        tags = results[c]["tags"]
        ts2 = results[c]["ts2"]
        ai, bi, l1, l2 = metas[c]
        for q in range(128):
            tag_seq[ai[q], : l1[q]] = tags[q, : l1[q]].astype(np.int32)
            tag_scores[ai[q]] = ts2[q, 0]
            tag_seq[bi[q], : l2[q]] = tags[q, l1[q]: l1[q] + l2[q]].astype(np.int32)
            tag_scores[bi[q]] = ts2[q, 1]
    return tag_seq, tag_scores


_LAST_RESULTS = {}
TRACE = False


def _ensure_ntff_hook():
    """Register the axon NTFF profiling hook (the image's antenv lacks
    axon_hooks; synthesize it from trn_agent_boot's ctypes shim)."""
    import sys as _sys
    import types as _types
    if "antenv.axon_hooks" in _sys.modules:
        return
    try:
        from trn_agent_boot.trn_boot import _ntff_profile_via_ctypes
        hook = _ntff_profile_via_ctypes("/opt/axon/libaxon_pjrt.so")
    except Exception:
        hook = None
    mod = _types.ModuleType("antenv.axon_hooks")
    mod._hook = hook
    mod.get_axon_ntff_profile_hook = lambda: mod._hook
    mod.set_axon_ntff_profile_hook = lambda h: setattr(mod, "_hook", h)
    _sys.modules["antenv.axon_hooks"] = mod


def kernel(feats, leng, transitions):
    feats = np.asarray(feats, np.float32)
    leng = np.asarray(leng, np.int32)
    transitions = np.asarray(transitions, np.float32)
    B, T_in, _ = feats.shape

    T, in_maps, metas = _host_prep(feats, leng, transitions)
    nc = build_program(T)
    if TRACE:
        _ensure_ntff_hook()
    res = bass_utils.run_bass_kernel_spmd(nc, in_maps,
                                          core_ids=list(range(len(in_maps))),
                                          trace=TRACE)
    _LAST_RESULTS["res"] = res
    return _host_post(res.results, metas, B, T_in)
